# revision 10
# baseline (speedup 1.0000x reference)
"""Trainium2 kernel for MinkLoc3D GeM pooling (segment_reduce).

Math:  out = L2norm_rows( (segment_mean(clip(x,1e-6)^p, batch_idx))^(1/p) )
with N=1e6 rows, C=256, B=16 segments, p=3.0, batch_idx sorted.

Strategy:
- batch_idx is sorted -> each segment is a contiguous row range. Assign 2
  whole segments to each of the 8 cores; every core runs an identical
  program on zero-padded per-segment buffers (zero rows contribute nothing
  to the sums). No collectives, no on-device batch_idx.
- Segment sums are row-permutation invariant, so the host hands each
  partition a contiguous chunk of rows: buffers are plain reshapes.
- Host casts f32 -> bf16 (halves HBM traffic; quantization error averages
  out over ~62k rows/segment). Device computes x^3 = (x^2)*x with two
  bf16 2x tensor_tensor passes on VectorE, then TensorE reduces rows via
  ones-vector matmuls (FD=512) accumulated in PSUM [1,512]; the host folds
  the two 256-halves. Keeping all elementwise work on one engine keeps
  every instruction at <=1 sync wait (this walrus build rejects more).
- counts / mean / ^(1/p) / L2-normalize run on host in float64 over the
  tiny (16,256) result.
"""

import math
from contextlib import ExitStack

import ml_dtypes
import numpy as np

NCORES = 8
G = 16  # 256-col chunks per DMA group; rows per group = 128*G
W = G * 256
SA = 3104  # columns of W squared on ScalarE (rest on VectorE); even
NACC = 4  # PSUM accumulators per segment (round-robin, pipelining)

last_results = None  # BassKernelResults of the most recent device run


def _split_excess_waits(nc):
    """This walrus build encodes at most ONE sync wait per instruction (two
    on EventSemaphore), but Tile's sem assignment happily emits more. Hoist
    the excess waits onto standalone EventSemaphore instructions inserted
    just before the over-subscribed instruction on the same engine queue —
    engine queues execute in order, so gating the queue is equivalent."""
    import concourse.mybir as mybir

    n_split = 0
    for f in nc.m.functions:
        for b in f.blocks:
            out_insts = []
            for i in b.instructions:
                si = i.sync_info
                waits = list(si.on_wait) if si and si.on_wait else []
                cap = 2 if isinstance(i, mybir.InstEventSemaphore) else 1
                if len(waits) > cap:
                    extra, keep = waits[:-cap], waits[-cap:]
                    for k in range(0, len(extra), 2):
                        n_split += 1
                        ev = mybir.InstEventSemaphore(
                            name=f"{i.name}-waitsplit-{k}",
                            engine=i.engine,
                            ins=[],
                            outs=[],
                        )
                        ev.sync_info = mybir.SyncInfo(
                            on_wait=extra[k : k + 2], on_update=[]
                        )
                        out_insts.append(ev)
                    i.sync_info = mybir.SyncInfo(
                        on_wait=keep, on_update=list(si.on_update or [])
                    )
                out_insts.append(i)
            b.instructions[:] = out_insts
    return n_split


def _build_nc(nG: int):
    import concourse.bass as bass
    import concourse.mybir as mybir
    import concourse.tile as tile

    nc = bass.Bass(name="gem_segsum")
    x = nc.dram_tensor(
        "x", [2, nG, 128, W], mybir.dt.bfloat16, kind="ExternalInput"
    )
    out = nc.dram_tensor(
        "out", [2, NACC, 512], mybir.dt.float32, kind="ExternalOutput"
    )

    nmm = W // 512
    with tile.TileContext(nc) as tc, ExitStack() as ctx:
        xp = ctx.enter_context(tc.tile_pool(name="xp", bufs=4))
        sqp = ctx.enter_context(tc.tile_pool(name="sqp", bufs=2))
        cbp = ctx.enter_context(tc.tile_pool(name="cbp", bufs=3))
        pp = ctx.enter_context(tc.tile_pool(name="pp", bufs=1, space="PSUM"))
        op = ctx.enter_context(tc.tile_pool(name="op", bufs=2))
        cp = ctx.enter_context(tc.tile_pool(name="cp", bufs=1))

        ones = cp.tile([128, 1], mybir.dt.bfloat16)
        nc.vector.memset(ones, 1.0)

        for s in range(2):
            # One full-bank PSUM tile per accumulator; accumulator j lives at
            # base partition 32*j so matmuls can round-robin PE column groups
            # (tile_position) — lets LDWEIGHTS pull ahead and sub-arrays
            # overlap instead of serializing on the same column group.
            banks = [
                pp.tile(
                    [128, 512], mybir.dt.float32, name=f"acc{s}{j}", tag=f"acc{s}{j}"
                )
                for j in range(NACC)
            ]
            accs = [banks[j][32 * j : 32 * j + 1, :] for j in range(NACC)]
            for g in range(nG):
                X = xp.tile([128, W], mybir.dt.bfloat16)
                nc.sync.dma_start(out=X[:, :], in_=x[s, g])
                SQ = sqp.tile([128, W], mybir.dt.bfloat16)
                if SA > 0:
                    nc.scalar.square(SQ[:, 0:SA], X[:, 0:SA])
                if SA < W:
                    nc.vector.tensor_mul(SQ[:, SA:W], X[:, SA:W], X[:, SA:W])
                CB = cbp.tile([128, W], mybir.dt.bfloat16)
                nc.vector.tensor_mul(CB[:, :], SQ[:, :], X[:, :])
                for k in range(nmm):
                    j = k % NACC
                    nc.tensor.matmul(
                        accs[j],
                        ones[:, :],
                        CB[:, k * 512 : (k + 1) * 512],
                        start=(g == 0 and k < NACC),
                        stop=(g == nG - 1 and k >= nmm - NACC),
                        tile_position=(0, 32 * j),
                    )
            for j in range(NACC):
                res = op.tile([1, 512], mybir.dt.float32)
                nc.vector.tensor_copy(res[:, :], accs[j])
                nc.sync.dma_start(out=out[s, j : j + 1, :], in_=res[:, :])
    _split_excess_waits(nc)
    return nc


_NC_CACHE = {}


def _device_segment_cube_sums(feats: np.ndarray, bounds: np.ndarray) -> np.ndarray:
    """Per-segment sums of x^3 on the 8 NeuronCores. feats f32 [N,256],
    bounds [17] row offsets of the 16 sorted segments. Returns f64 [16,256]."""
    from concourse.bass_utils import run_bass_kernel_spmd

    global last_results

    if feats.min() < 0.0:
        feats = np.maximum(feats, 1e-6)
    xbf = feats.astype(ml_dtypes.bfloat16)

    seg_rows = np.diff(bounds)
    rows_per_group = 128 * G
    nG = max(1, math.ceil(int(seg_rows.max()) / rows_per_group))
    r_pad = nG * rows_per_group

    in_maps = []
    for i in range(NCORES):
        buf = np.zeros((2, r_pad, 256), dtype=ml_dtypes.bfloat16)
        for s in range(2):
            seg = 2 * i + s
            r0, r1 = int(bounds[seg]), int(bounds[seg + 1])
            buf[s, : r1 - r0] = xbf[r0:r1]
        in_maps.append({"x": buf.reshape(2, nG, 128, W)})

    if nG not in _NC_CACHE:
        _NC_CACHE[nG] = _build_nc(nG)
    nc = _NC_CACHE[nG]

    last_results = run_bass_kernel_spmd(nc, in_maps, core_ids=list(range(NCORES)))
    parts = np.stack(
        [last_results.results[i]["out"] for i in range(NCORES)], axis=0
    ).astype(np.float64)  # [NCORES, 2, NACC, 512]
    halves = parts.sum(axis=2)  # fold round-robin accumulators
    sums = halves[:, :, :256] + halves[:, :, 256:]  # fold even/odd chunks
    return sums.reshape(2 * NCORES, 256)


def _fallback_segment_pow_sums(
    feats: np.ndarray, bounds: np.ndarray, B: int, pval: float
) -> np.ndarray:
    """Pure-numpy reference path for unexpected shapes/p. f64 [B,C]."""
    xp = np.clip(feats.astype(np.float64), 1e-6, None) ** pval
    sums = np.zeros((B, xp.shape[1]), dtype=np.float64)
    for s in range(B):
        sums[s] = xp[bounds[s] : bounds[s + 1]].sum(axis=0)
    return sums


def kernel(features, p, batch_idx, num_batches):
    feats = np.ascontiguousarray(np.asarray(features, dtype=np.float32))
    bidx = np.asarray(batch_idx)
    B = int(np.asarray(num_batches))
    pval = float(np.asarray(p, dtype=np.float64).reshape(-1)[0])
    N, C = feats.shape

    if not np.all(bidx[1:] >= bidx[:-1]):
        order = np.argsort(bidx, kind="stable")
        feats = feats[order]
        bidx = bidx[order]
    bounds = np.searchsorted(bidx, np.arange(B + 1))
    counts = np.diff(bounds).astype(np.float64)

    if pval == 3.0 and C == 256 and B == 2 * NCORES:
        sums = _device_segment_cube_sums(feats, bounds)
    else:
        sums = _fallback_segment_pow_sums(feats, bounds, B, pval)

    with np.errstate(divide="ignore", invalid="ignore"):
        mean = sums / counts[:, None]
        desc = np.power(mean, 1.0 / pval)
        norm = np.sqrt((desc * desc).sum(axis=1, keepdims=True))
        out = desc / np.maximum(norm, 1e-12)
    return out.astype(np.float32)


# revision 11
# speedup vs baseline: 1.1162x; 1.1162x over previous
"""Trainium2 kernel for MinkLoc3D GeM pooling (segment_reduce).

Math:  out = L2norm_rows( (segment_mean(clip(x,1e-6)^p, batch_idx))^(1/p) )
with N=1e6 rows, C=256, B=16 segments, p=3.0, batch_idx sorted.

Strategy:
- batch_idx is sorted -> each segment is a contiguous row range. Assign 2
  whole segments to each of the 8 cores; every core runs an identical
  program on zero-padded per-segment buffers (zero rows contribute nothing
  to the sums). No collectives, no on-device batch_idx.
- Segment sums are row-permutation invariant, so the host hands each
  partition a contiguous chunk of rows: buffers are plain reshapes.
- Host casts f32 -> bf16 (halves HBM traffic; quantization error averages
  out over ~62k rows/segment). Device computes x^3 = (x^2)*x with two
  bf16 2x tensor_tensor passes on VectorE, then TensorE reduces rows via
  ones-vector matmuls (FD=512) accumulated in PSUM [1,512]; the host folds
  the two 256-halves. Keeping all elementwise work on one engine keeps
  every instruction at <=1 sync wait (this walrus build rejects more).
- counts / mean / ^(1/p) / L2-normalize run on host in float64 over the
  tiny (16,256) result.
"""

import math
from contextlib import ExitStack

import ml_dtypes
import numpy as np

NCORES = 8
G = 16  # 256-col chunks per DMA group; rows per group = 128*G
W = G * 256
SA = 3104  # columns of W squared on ScalarE (rest on VectorE); even
NACC = 4  # PSUM accumulators per segment (round-robin, pipelining)

last_results = None  # BassKernelResults of the most recent device run


def _split_excess_waits(nc):
    """This walrus build encodes at most ONE sync wait per instruction (two
    on EventSemaphore), but Tile's sem assignment happily emits more. Hoist
    the excess waits onto standalone EventSemaphore instructions inserted
    just before the over-subscribed instruction on the same engine queue —
    engine queues execute in order, so gating the queue is equivalent."""
    import concourse.mybir as mybir

    n_split = 0
    for f in nc.m.functions:
        for b in f.blocks:
            out_insts = []
            for i in b.instructions:
                si = i.sync_info
                waits = list(si.on_wait) if si and si.on_wait else []
                cap = 2 if isinstance(i, mybir.InstEventSemaphore) else 1
                if len(waits) > cap:
                    extra, keep = waits[:-cap], waits[-cap:]
                    for k in range(0, len(extra), 2):
                        n_split += 1
                        ev = mybir.InstEventSemaphore(
                            name=f"{i.name}-waitsplit-{k}",
                            engine=i.engine,
                            ins=[],
                            outs=[],
                        )
                        ev.sync_info = mybir.SyncInfo(
                            on_wait=extra[k : k + 2], on_update=[]
                        )
                        out_insts.append(ev)
                    i.sync_info = mybir.SyncInfo(
                        on_wait=keep, on_update=list(si.on_update or [])
                    )
                out_insts.append(i)
            b.instructions[:] = out_insts
    return n_split


def _build_nc(nG: int):
    import concourse.bass as bass
    import concourse.mybir as mybir
    import concourse.tile as tile

    nc = bass.Bass(name="gem_segsum")
    x = nc.dram_tensor(
        "x", [2, nG, 128, W], mybir.dt.bfloat16, kind="ExternalInput"
    )
    out = nc.dram_tensor(
        "out", [2, NACC, 512], mybir.dt.float32, kind="ExternalOutput"
    )

    nmm = W // 512
    with tile.TileContext(nc) as tc, ExitStack() as ctx:
        xp = ctx.enter_context(tc.tile_pool(name="xp", bufs=4))
        sqp = ctx.enter_context(tc.tile_pool(name="sqp", bufs=2))
        cbp = ctx.enter_context(tc.tile_pool(name="cbp", bufs=3))
        pp = ctx.enter_context(tc.tile_pool(name="pp", bufs=1, space="PSUM"))
        op = ctx.enter_context(tc.tile_pool(name="op", bufs=2))
        cp = ctx.enter_context(tc.tile_pool(name="cp", bufs=1))

        ones = cp.tile([128, 1], mybir.dt.bfloat16)
        nc.vector.memset(ones, 1.0)

        for s in range(2):
            # One full-bank PSUM tile per accumulator; accumulator j lives at
            # base partition 32*j so matmuls can round-robin PE column groups
            # (tile_position) — lets LDWEIGHTS pull ahead and sub-arrays
            # overlap instead of serializing on the same column group.
            banks = [
                pp.tile(
                    [128, 512], mybir.dt.float32, name=f"acc{s}{j}", tag=f"acc{s}{j}"
                )
                for j in range(NACC)
            ]
            accs = [banks[j][0:1, :] for j in range(NACC)]
            for g in range(nG):
                X = xp.tile([128, W], mybir.dt.bfloat16)
                nc.sync.dma_start(out=X[:, :], in_=x[s, g])
                SQ = sqp.tile([128, W], mybir.dt.bfloat16)
                if SA > 0:
                    nc.scalar.square(SQ[:, 0:SA], X[:, 0:SA])
                if SA < W:
                    nc.vector.tensor_mul(SQ[:, SA:W], X[:, SA:W], X[:, SA:W])
                CB = cbp.tile([128, W], mybir.dt.bfloat16)
                nc.vector.tensor_mul(CB[:, :], SQ[:, :], X[:, :])
                for k in range(nmm):
                    j = k % NACC
                    nc.tensor.matmul(
                        accs[j],
                        ones[:, :],
                        CB[:, k * 512 : (k + 1) * 512],
                        start=(g == 0 and k < NACC),
                        stop=(g == nG - 1 and k >= nmm - NACC),
                    )
            for j in range(NACC):
                res = op.tile([1, 512], mybir.dt.float32)
                nc.vector.tensor_copy(res[:, :], accs[j])
                nc.sync.dma_start(out=out[s, j : j + 1, :], in_=res[:, :])
    _split_excess_waits(nc)
    return nc


_NC_CACHE = {}


def _device_segment_cube_sums(feats: np.ndarray, bounds: np.ndarray) -> np.ndarray:
    """Per-segment sums of x^3 on the 8 NeuronCores. feats f32 [N,256],
    bounds [17] row offsets of the 16 sorted segments. Returns f64 [16,256]."""
    from concourse.bass_utils import run_bass_kernel_spmd

    global last_results

    if feats.min() < 0.0:
        feats = np.maximum(feats, 1e-6)
    xbf = feats.astype(ml_dtypes.bfloat16)

    seg_rows = np.diff(bounds)
    rows_per_group = 128 * G
    nG = max(1, math.ceil(int(seg_rows.max()) / rows_per_group))
    r_pad = nG * rows_per_group

    in_maps = []
    for i in range(NCORES):
        buf = np.zeros((2, r_pad, 256), dtype=ml_dtypes.bfloat16)
        for s in range(2):
            seg = 2 * i + s
            r0, r1 = int(bounds[seg]), int(bounds[seg + 1])
            buf[s, : r1 - r0] = xbf[r0:r1]
        in_maps.append({"x": buf.reshape(2, nG, 128, W)})

    if nG not in _NC_CACHE:
        _NC_CACHE[nG] = _build_nc(nG)
    nc = _NC_CACHE[nG]

    last_results = run_bass_kernel_spmd(nc, in_maps, core_ids=list(range(NCORES)))
    parts = np.stack(
        [last_results.results[i]["out"] for i in range(NCORES)], axis=0
    ).astype(np.float64)  # [NCORES, 2, NACC, 512]
    halves = parts.sum(axis=2)  # fold round-robin accumulators
    sums = halves[:, :, :256] + halves[:, :, 256:]  # fold even/odd chunks
    return sums.reshape(2 * NCORES, 256)


def _fallback_segment_pow_sums(
    feats: np.ndarray, bounds: np.ndarray, B: int, pval: float
) -> np.ndarray:
    """Pure-numpy reference path for unexpected shapes/p. f64 [B,C]."""
    xp = np.clip(feats.astype(np.float64), 1e-6, None) ** pval
    sums = np.zeros((B, xp.shape[1]), dtype=np.float64)
    for s in range(B):
        sums[s] = xp[bounds[s] : bounds[s + 1]].sum(axis=0)
    return sums


def kernel(features, p, batch_idx, num_batches):
    feats = np.ascontiguousarray(np.asarray(features, dtype=np.float32))
    bidx = np.asarray(batch_idx)
    B = int(np.asarray(num_batches))
    pval = float(np.asarray(p, dtype=np.float64).reshape(-1)[0])
    N, C = feats.shape

    if not np.all(bidx[1:] >= bidx[:-1]):
        order = np.argsort(bidx, kind="stable")
        feats = feats[order]
        bidx = bidx[order]
    bounds = np.searchsorted(bidx, np.arange(B + 1))
    counts = np.diff(bounds).astype(np.float64)

    if pval == 3.0 and C == 256 and B == 2 * NCORES:
        sums = _device_segment_cube_sums(feats, bounds)
    else:
        sums = _fallback_segment_pow_sums(feats, bounds, B, pval)

    with np.errstate(divide="ignore", invalid="ignore"):
        mean = sums / counts[:, None]
        desc = np.power(mean, 1.0 / pval)
        norm = np.sqrt((desc * desc).sum(axis=1, keepdims=True))
        out = desc / np.maximum(norm, 1e-12)
    return out.astype(np.float32)
